# revision 8
# baseline (speedup 1.0000x reference)
"""LEM cell (ODE2) Bass kernel for Trainium2, 8-core data-parallel, fp8.

Math (per batch row b):
  ti = x @ W_ih.T + b_ih                  # [B, 4H]
  th = y @ W_hh.T + b_hh                  # [B, 3H]
  tdt = dt @ W_dt.T + b_dt                # [B, 2]
  ms_dt_bar = sig(tdt[:,0]) * sig(ti[:, :H]   + th[:, :H])
  ms_dt     = sig(tdt[:,1]) * sig(ti[:, H:2H] + th[:, H:2H])
  z_new = (1-ms_dt) * z + ms_dt * tanh(ti[:, 3H:] + th[:, 2H:3H])
  y_new = (1-ms_dt_bar) * y + ms_dt_bar * tanh(z_new @ W_z.T + b_z + ti[:, 2H:3H])
  returns (y_new, z_new)

Strategy: shard batch across 8 cores (2048 rows each); feature-major on-chip
layout ([128-partition feature tile, batch columns free]).  All eight H^2
GEMM-units run as fp8e4m3 DoubleRow matmuls (2x PE rate): host quantizes
x/y and the weight packs (scaled by 32, folded back via the activation
scale=1/32), z_new is cast to fp8 on-chip for the W_z GEMM.  The i+h sums
and i_z + z_new@W_z.T are accumulated in PSUM across both operand groups.
Pointwise runs in bf16 (2x DVE rate); z/y pointwise inputs and both outputs
travel as bf16.  Whole 2048-row shard is one panel; all 32 weight blocks
(64KB/partition) stay resident; each stationary weight block serves 4
chunk-columns back to back to amortize PE weight loads.
"""

import sys

_REPO = "/opt/trn_rl_repo"
if _REPO not in sys.path:
    sys.path.insert(0, _REPO)

from contextlib import ExitStack

import numpy as np
import ml_dtypes

import concourse.bacc as bacc
import concourse.tile as tile
from concourse import mybir
from concourse.bass_utils import run_bass_kernel_spmd

P = 128
F32 = mybir.dt.float32
BF16 = mybir.dt.bfloat16
FP8 = mybir.dt.float8e4
AF = mybir.ActivationFunctionType
PM = mybir.MatmulPerfMode

E4NP = ml_dtypes.float8_e4m3
BFNP = ml_dtypes.bfloat16

N_CORES = 8
NINP = 1024
NHID = 1024
BATCH = 16384
WSCALE = 32.0  # weight pre-scale before fp8 quantization (power of 2)

LAST_RESULTS = None  # BassKernelResults of the most recent kernel() call


def build_nc(
    K,            # input feature dim (x)
    H,            # hidden dim (y/z)
    B_shard,      # batch rows per core
    chunk,        # matmul moving-dim size (psum free size, <=512)
    wdt00, wdt10,  # W_dt scalars (baked immediates; b_dt rides in biasP)
):
    NJT = H // P           # output feature tiles (per H-sized group)
    NKT = K // P           # x contraction 128-blocks
    NHT = H // P           # y/z contraction 128-blocks
    NKP = NKT // 2         # x contraction 256-pairs (DoubleRow)
    NHP = NHT // 2
    nch = B_shard // chunk

    nc = bacc.Bacc(trn_type="TRN2", target_bir_lowering=False)

    x8 = nc.declare_dram_parameter("x8", [K // 2, 2, B_shard], FP8, isOutput=False)
    y8 = nc.declare_dram_parameter("y8", [H // 2, 2, B_shard], FP8, isOutput=False)
    ybf = nc.declare_dram_parameter("ybf", [H, B_shard], BF16, isOutput=False)
    zbf = nc.declare_dram_parameter("zbf", [H, B_shard], BF16, isOutput=False)
    dtr = nc.declare_dram_parameter("dtr", [1, B_shard], F32, isOutput=False)
    # packed stationary blocks: [jt, kin, kidx, j]; kidx 0..NKT-1 = x features,
    # NKT..NKT+NHT-1 = y (or z_new) features; values are fp8(32*W)
    NKH = NKT + NHT
    Wd2 = nc.declare_dram_parameter("Wd2", [NJT, P, NKH, P], FP8, isOutput=False)
    Wy = nc.declare_dram_parameter("Wy", [NJT, P, NKH, P], FP8, isOutput=False)
    Wd1 = nc.declare_dram_parameter("Wd1", [NJT, P, NKH, P], FP8, isOutput=False)
    Wg3 = nc.declare_dram_parameter("Wg3", [NJT, P, NKH, P], FP8, isOutput=False)
    # last two columns: row 0 holds b_dt[0], b_dt[1]
    biasP = nc.declare_dram_parameter("biasP", [P, 4 * NJT + 2], F32, isOutput=False)

    y_newT = nc.declare_dram_parameter("y_newT", [H, B_shard], F32, isOutput=True)
    z_newT = nc.declare_dram_parameter("z_newT", [H, B_shard], F32, isOutput=True)

    with tile.TileContext(nc) as tc, ExitStack() as ctx:
        dchunk = 2 * chunk          # DVE op width (2 psum chunks)
        ndc = B_shard // dchunk
        cpool = ctx.enter_context(tc.tile_pool(name="cpool", bufs=1))
        wpool = ctx.enter_context(tc.tile_pool(name="wpool", bufs=1))
        x8pool = ctx.enter_context(tc.tile_pool(name="x8pool", bufs=NKP))
        y8pool = ctx.enter_context(tc.tile_pool(name="y8pool", bufs=NHP))
        zpool = ctx.enter_context(tc.tile_pool(name="zpool", bufs=2))
        ypool = ctx.enter_context(tc.tile_pool(name="ypool", bufs=2))
        bcpool = ctx.enter_context(tc.tile_pool(name="bcpool", bufs=1))
        rpool = ctx.enter_context(tc.tile_pool(name="rpool", bufs=1))
        apool = ctx.enter_context(tc.tile_pool(name="apool", bufs=3))
        dpool = ctx.enter_context(tc.tile_pool(name="dpool", bufs=3))
        opool = ctx.enter_context(tc.tile_pool(name="opool", bufs=3))
        znpool = ctx.enter_context(tc.tile_pool(name="znpool", bufs=1))
        pspool = ctx.enter_context(tc.tile_pool(name="pspool", bufs=8, space="PSUM"))

        bias_sb = cpool.tile([P, 4 * NJT + 2], F32, name="bias_sb")
        nc.sync.dma_start(bias_sb[:], biasP[:, :])

        def bias_ap(g, jt):
            i = g * NJT + jt
            return bias_sb[:, i : i + 1]

        def cs(c):
            return slice(c * chunk, (c + 1) * chunk)

        def ds(c2):
            return slice(c2 * dchunk, (c2 + 1) * dchunk)

        # per-batch dt gates first: tiny ACT ops must precede the PSUM
        # activations in the ACT FIFO so the bc gates are ready early
        dt_sb = rpool.tile([1, B_shard], F32, name="dt_sb")
        nc.sync.dma_start(dt_sb[:], dtr[0:1, :])
        sg1 = rpool.tile([1, B_shard], BF16, name="sg1")
        nc.scalar.activation(
            sg1[:], dt_sb[:], AF.Sigmoid,
            bias=bias_sb[0:1, 4 * NJT : 4 * NJT + 1], scale=wdt00,
        )
        sg2 = rpool.tile([1, B_shard], BF16, name="sg2")
        nc.scalar.activation(
            sg2[:], dt_sb[:], AF.Sigmoid,
            bias=bias_sb[0:1, 4 * NJT + 1 : 4 * NJT + 2], scale=wdt10,
        )
        y_t = []
        for g in range(NHP):
            yt_ = y8pool.tile([P, 2, B_shard], FP8, name="yt", tag="yt")
            if g < 2:
                nc.gpsimd.dma_start(yt_[:], y8[g * P : (g + 1) * P, :, :])
            y_t.append(yt_)
        bc1 = bcpool.tile([P, B_shard], BF16, name="bc1")
        nc.gpsimd.partition_broadcast(bc1[:], sg1[0:1, :])
        bc2 = bcpool.tile([P, B_shard], BF16, name="bc2")
        nc.gpsimd.partition_broadcast(bc2[:], sg2[0:1, :])

        # ---- resident loads ----
        # phase-B weights + x8 g2/g3 on sync, x8 g0/g1 + y8 g2/g3 on scalar,
        # y8 g0/g1 on gpsimd: spreads the ~4MB cold-start load across queues.
        x_t_late = {}
        for g in (2, 3):
            x_t_late[g] = x8pool.tile([P, 2, B_shard], FP8, name="xt", tag="xt")
        w_d2, w_y, w_d1, w_g3 = [], [], [], []
        x_t = []
        for jt in range(NJT):
            wt = wpool.tile([P, NKH, P], FP8, name=f"wd2_{jt}", tag=f"wd2_{jt}")
            nc.sync.dma_start(wt[:], Wd2[jt][:, :, :])
            w_d2.append(wt)
            wt = wpool.tile([P, NKH, P], FP8, name=f"wy_{jt}", tag=f"wy_{jt}")
            nc.sync.dma_start(wt[:], Wy[jt][:, :, :])
            w_y.append(wt)
            if jt < 2:
                xt_ = x8pool.tile([P, 2, B_shard], FP8, name="xt", tag="xt")
                nc.scalar.dma_start(xt_[:], x8[jt * P : (jt + 1) * P, :, :])
                x_t.append(xt_)
            elif jt < NKP:
                x_t.append(x_t_late[jt])
            if jt == 0:
                # cold-start spread: x8 g2/g3 ride the sync queue right after
                # jt0 weights; y8 g2/g3 follow x8 g0/g1 on scalar
                for g in (2, 3):
                    nc.sync.dma_start(x_t_late[g][:], x8[g * P : (g + 1) * P, :, :])
                for g in (2, 3):
                    nc.scalar.dma_start(y_t[g][:], y8[g * P : (g + 1) * P, :, :])
        # phase-C weights stream on gpsimd during phase B
        for jt in range(NJT):
            wt = wpool.tile([P, NKH, P], FP8, name=f"wd1_{jt}", tag=f"wd1_{jt}")
            nc.gpsimd.dma_start(wt[:], Wd1[jt][:, :, :])
            w_d1.append(wt)
            wt = wpool.tile([P, NKH, P], FP8, name=f"wg3_{jt}", tag=f"wg3_{jt}")
            nc.gpsimd.dma_start(wt[:], Wg3[jt][:, :, :])
            w_g3.append(wt)

        # fp8 z_new, resident for the W_z GEMM: [kin, hidx, batch]
        zn8 = znpool.tile([P, NHT, B_shard], FP8, name="zn8")

        def accum_group(ps_tiles, w_sb, rhs_a, rhs_b):
            """ps[c] = sum_g Wa[g].T@a[g][c] + Wb[g].T@b[g][c], DoubleRow.

            g-major / c-minor order so each stationary block is loaded once
            per nch moving matmuls."""
            n_a = len(rhs_a)
            n_b = len(rhs_b)
            for g in range(n_a):
                lhsT = w_sb[:, 2 * g : 2 * g + 2, :]
                for c in range(len(ps_tiles)):
                    nc.tensor.matmul(
                        ps_tiles[c][:], lhsT=lhsT, rhs=rhs_a[g][c],
                        start=(g == 0), stop=False, perf_mode=PM.DoubleRow,
                    )
            for g in range(n_b):
                lhsT = w_sb[:, NKT + 2 * g : NKT + 2 * g + 2, :]
                for c in range(len(ps_tiles)):
                    nc.tensor.matmul(
                        ps_tiles[c][:], lhsT=lhsT, rhs=rhs_b[g][c],
                        start=False, stop=(g == n_b - 1), perf_mode=PM.DoubleRow,
                    )

        def xrhs(g):
            return [x_t[g][:, :, cs(c)] for c in range(nch)]

        def yrhs(g):
            return [y_t[g][:, :, cs(c)] for c in range(nch)]

        # ---- phase B: d2 + y gates -> z_new ----
        for jt in range(NJT):
            jp = slice(jt * P, (jt + 1) * P)
            z16 = zpool.tile([P, B_shard], BF16, name="z16", tag="z")
            nc.gpsimd.dma_start(z16[:], zbf[jp, :])

            ps1 = [pspool.tile([P, chunk], F32, name="ps1", tag="ps") for _ in range(nch)]
            accum_group(ps1, w_d2[jt],
                        [xrhs(g) for g in range(NKP)], [yrhs(g) for g in range(NHP)])
            s2, gm = [], []
            for c2 in range(ndc):
                t = apool.tile([P, dchunk], F32, name="s2", tag="sg", bufs=2)
                nc.scalar.activation(t[:, 0:chunk], ps1[2 * c2][:], AF.Sigmoid,
                                     bias=bias_ap(0, jt), scale=1.0 / WSCALE)
                nc.scalar.activation(t[:, chunk:dchunk], ps1[2 * c2 + 1][:], AF.Sigmoid,
                                     bias=bias_ap(0, jt), scale=1.0 / WSCALE)
                s2.append(t)
                # gm only needs s2 -> issue before the second matmul sweep drains
                g_ = dpool.tile([P, dchunk], F32, name="gm", tag="gm", bufs=2)
                nc.vector.tensor_mul(g_[:], t[:], bc2[:, ds(c2)])
                gm.append(g_)

            ps2 = [pspool.tile([P, chunk], F32, name="ps2", tag="ps") for _ in range(nch)]
            accum_group(ps2, w_y[jt],
                        [xrhs(g) for g in range(NKP)], [yrhs(g) for g in range(NHP)])
            for c2 in range(ndc):
                tz = apool.tile([P, dchunk], F32, name="tz", tag="th", bufs=2)
                nc.scalar.activation(tz[:, 0:chunk], ps2[2 * c2][:], AF.Tanh,
                                     bias=bias_ap(1, jt), scale=1.0 / WSCALE)
                nc.scalar.activation(tz[:, chunk:dchunk], ps2[2 * c2 + 1][:], AF.Tanh,
                                     bias=bias_ap(1, jt), scale=1.0 / WSCALE)
                d = dpool.tile([P, dchunk], F32, name="d", tag="dm", bufs=2)
                nc.vector.tensor_sub(d[:], tz[:], z16[:, ds(c2)])
                m = dpool.tile([P, dchunk], F32, name="m", tag="mm", bufs=2)
                nc.vector.tensor_mul(m[:], gm[c2][:], d[:])
                znc = opool.tile([P, dchunk], F32, name="znc", tag="on")
                nc.vector.tensor_add(znc[:], m[:], z16[:, ds(c2)])
                nc.sync.dma_start(z_newT[jp, ds(c2)], znc[:])
                # fp8 cast into the resident zn8 for the W_z GEMM
                nc.scalar.activation(zn8[:, jt, ds(c2)], znc[:], AF.Copy)

        # ---- phase C: d1 gate + (i_z + z_new @ W_z.T) -> y_new ----
        for jt in range(NJT):
            jp = slice(jt * P, (jt + 1) * P)
            y16 = ypool.tile([P, B_shard], BF16, name="y16", tag="y")
            nc.gpsimd.dma_start(y16[:], ybf[jp, :])

            ps3 = [pspool.tile([P, chunk], F32, name="ps3", tag="ps") for _ in range(nch)]
            accum_group(ps3, w_d1[jt],
                        [xrhs(g) for g in range(NKP)], [yrhs(g) for g in range(NHP)])
            s1, gm1 = [], []
            for c2 in range(ndc):
                t = apool.tile([P, dchunk], F32, name="s1", tag="sg", bufs=2)
                nc.scalar.activation(t[:, 0:chunk], ps3[2 * c2][:], AF.Sigmoid,
                                     bias=bias_ap(2, jt), scale=1.0 / WSCALE)
                nc.scalar.activation(t[:, chunk:dchunk], ps3[2 * c2 + 1][:], AF.Sigmoid,
                                     bias=bias_ap(2, jt), scale=1.0 / WSCALE)
                s1.append(t)
                g_ = dpool.tile([P, dchunk], F32, name="gm1", tag="gm", bufs=2)
                nc.vector.tensor_mul(g_[:], t[:], bc1[:, ds(c2)])
                gm1.append(g_)

            ps4 = [pspool.tile([P, chunk], F32, name="ps4", tag="ps") for _ in range(nch)]
            accum_group(ps4, w_g3[jt],
                        [xrhs(g) for g in range(NKP)],
                        [[zn8[:, 2 * g : 2 * g + 2, cs(c)] for c in range(nch)]
                         for g in range(NHP)])
            for c2 in range(ndc):
                u = apool.tile([P, dchunk], F32, name="u", tag="th", bufs=2)
                nc.scalar.activation(u[:, 0:chunk], ps4[2 * c2][:], AF.Tanh,
                                     bias=bias_ap(3, jt), scale=1.0 / WSCALE)
                nc.scalar.activation(u[:, chunk:dchunk], ps4[2 * c2 + 1][:], AF.Tanh,
                                     bias=bias_ap(3, jt), scale=1.0 / WSCALE)
                d = dpool.tile([P, dchunk], F32, name="dy", tag="dm", bufs=2)
                nc.vector.tensor_sub(d[:], u[:], y16[:, ds(c2)])
                m = dpool.tile([P, dchunk], F32, name="my", tag="mm", bufs=2)
                nc.vector.tensor_mul(m[:], gm1[c2][:], d[:])
                yn = opool.tile([P, dchunk], F32, name="yn", tag="on")
                nc.vector.tensor_add(yn[:], m[:], y16[:, ds(c2)])
                nc.scalar.dma_start(y_newT[jp, ds(c2)], yn[:])

    nc.compile()
    return nc


def _pack_pair_fp8(Wa, Wb):
    """[jt, kin, kidx, j] stationary-block packing of two row-major [out, in]
    weight matrices, quantized to fp8(32*W)."""
    def pack(W):
        O, I = W.shape
        njt, nkt = O // P, I // P
        Wq = np.asarray(W * WSCALE, dtype=E4NP)
        # [jt, j, kt, kin] -> [jt, kin, kt, j]
        return Wq.reshape(njt, P, nkt, P).transpose(0, 3, 2, 1)
    return np.ascontiguousarray(np.concatenate([pack(Wa), pack(Wb)], axis=2))


def _pack_act_fp8(aT):
    """[K, B] fp8 -> [K//2, 2, B] DoubleRow pair-major packing."""
    Kdim, B = aT.shape
    nkp = Kdim // (2 * P)
    return np.ascontiguousarray(
        aT.reshape(nkp, 2, P, B).transpose(0, 2, 1, 3).reshape(Kdim // 2, 2, B)
    )


def pack_host_inputs(x, y, z, dt, W_ih, b_ih, W_hh, b_hh, W_z, b_z, b_dt, n_cores):
    """Shard batch across cores; quantize + pre-transpose activations;
    pack weights."""
    B, K = x.shape
    H = y.shape[1]
    NJT = H // P
    Bs = B // n_cores

    x8 = _pack_act_fp8(np.ascontiguousarray(np.asarray(x, dtype=E4NP).T))
    y8 = _pack_act_fp8(np.ascontiguousarray(np.asarray(y, dtype=E4NP).T))
    ybf = np.ascontiguousarray(np.asarray(y, dtype=BFNP).T)
    zbf = np.ascontiguousarray(np.asarray(z, dtype=BFNP).T)
    dtrow = np.ascontiguousarray(dt.reshape(1, B))

    Wd2 = _pack_pair_fp8(W_ih[H : 2 * H], W_hh[H : 2 * H])
    Wy = _pack_pair_fp8(W_ih[3 * H : 4 * H], W_hh[2 * H : 3 * H])
    Wd1 = _pack_pair_fp8(W_ih[0:H], W_hh[0:H])
    Wg3 = _pack_pair_fp8(W_ih[2 * H : 3 * H], W_z)

    def bias_cols(bvec):
        return bvec.reshape(NJT, P).T  # [P, NJT]

    bdt_cols = np.zeros((P, 2), np.float32)
    bdt_cols[0, 0] = b_dt[0]
    bdt_cols[0, 1] = b_dt[1]
    biasP = np.ascontiguousarray(
        np.concatenate(
            [
                bias_cols(b_ih[H : 2 * H] + b_hh[H : 2 * H]),
                bias_cols(b_ih[3 * H : 4 * H] + b_hh[2 * H : 3 * H]),
                bias_cols(b_ih[0:H] + b_hh[0:H]),
                bias_cols(b_ih[2 * H : 3 * H] + b_z),
                bdt_cols,
            ],
            axis=1,
        ),
        dtype=np.float32,
    )

    in_maps = []
    for c in range(n_cores):
        sl = slice(c * Bs, (c + 1) * Bs)
        in_maps.append(
            {
                "x8": np.ascontiguousarray(x8[:, :, sl]),
                "y8": np.ascontiguousarray(y8[:, :, sl]),
                "ybf": np.ascontiguousarray(ybf[:, sl]),
                "zbf": np.ascontiguousarray(zbf[:, sl]),
                "dtr": np.ascontiguousarray(dtrow[:, sl]),
                "Wd2": Wd2,
                "Wy": Wy,
                "Wd1": Wd1,
                "Wg3": Wg3,
                "biasP": biasP,
            }
        )
    return in_maps


def kernel(x, y, z, dt, W_ih, b_ih, W_hh, b_hh, W_z, b_z, W_dt, b_dt):
    x = np.asarray(x, np.float32)
    y = np.asarray(y, np.float32)
    z = np.asarray(z, np.float32)
    dt = np.asarray(dt, np.float32)
    W_ih = np.asarray(W_ih, np.float32)
    b_ih = np.asarray(b_ih, np.float32)
    W_hh = np.asarray(W_hh, np.float32)
    b_hh = np.asarray(b_hh, np.float32)
    W_z = np.asarray(W_z, np.float32)
    b_z = np.asarray(b_z, np.float32)
    W_dt = np.asarray(W_dt, np.float32)
    b_dt = np.asarray(b_dt, np.float32)

    B, K = x.shape
    H = y.shape[1]
    Bs = B // N_CORES

    in_maps = pack_host_inputs(
        x, y, z, dt, W_ih, b_ih, W_hh, b_hh, W_z, b_z, b_dt, N_CORES
    )
    nc = build_nc(
        K,
        H,
        Bs,
        chunk=512,
        wdt00=float(W_dt[0, 0]),
        wdt10=float(W_dt[1, 0]),
    )
    import os

    trace = os.environ.get("LEM_TRACE", "0") == "1"
    tmpdir = os.environ.get("LEM_TMPDIR") or None
    res = run_bass_kernel_spmd(
        nc, in_maps, list(range(N_CORES)), trace=trace, tmpdir=tmpdir
    )
    global LAST_RESULTS
    LAST_RESULTS = res
    y_newT = np.concatenate([np.asarray(r["y_newT"]) for r in res.results], axis=1)
    z_newT = np.concatenate([np.asarray(r["z_newT"]) for r in res.results], axis=1)
    return (
        np.ascontiguousarray(y_newT.T, dtype=np.float32),
        np.ascontiguousarray(z_newT.T, dtype=np.float32),
    )


# revision 9
# speedup vs baseline: 1.0403x; 1.0403x over previous
"""LEM cell (ODE2) Bass kernel for Trainium2, 8-core data-parallel, fp8.

Math (per batch row b):
  ti = x @ W_ih.T + b_ih                  # [B, 4H]
  th = y @ W_hh.T + b_hh                  # [B, 3H]
  tdt = dt @ W_dt.T + b_dt                # [B, 2]
  ms_dt_bar = sig(tdt[:,0]) * sig(ti[:, :H]   + th[:, :H])
  ms_dt     = sig(tdt[:,1]) * sig(ti[:, H:2H] + th[:, H:2H])
  z_new = (1-ms_dt) * z + ms_dt * tanh(ti[:, 3H:] + th[:, 2H:3H])
  y_new = (1-ms_dt_bar) * y + ms_dt_bar * tanh(z_new @ W_z.T + b_z + ti[:, 2H:3H])
  returns (y_new, z_new)

Strategy: shard batch across 8 cores (2048 rows each); feature-major on-chip
layout ([128-partition feature tile, batch columns free]).  All eight H^2
GEMM-units run as fp8e4m3 DoubleRow matmuls (2x PE rate): host quantizes
x/y and the weight packs (scaled by 32, folded back via the activation
scale=1/32), z_new is cast to fp8 on-chip for the W_z GEMM.  The i+h sums
and i_z + z_new@W_z.T are accumulated in PSUM across both operand groups.
Pointwise runs in bf16 (2x DVE rate); z/y pointwise inputs and both outputs
travel as bf16.  Whole 2048-row shard is one panel; all 32 weight blocks
(64KB/partition) stay resident; each stationary weight block serves 4
chunk-columns back to back to amortize PE weight loads.
"""

import sys

_REPO = "/opt/trn_rl_repo"
if _REPO not in sys.path:
    sys.path.insert(0, _REPO)

from contextlib import ExitStack

import numpy as np
import ml_dtypes

import concourse.bacc as bacc
import concourse.tile as tile
from concourse import mybir
from concourse.bass_utils import run_bass_kernel_spmd

P = 128
F32 = mybir.dt.float32
BF16 = mybir.dt.bfloat16
FP8 = mybir.dt.float8e4
AF = mybir.ActivationFunctionType
PM = mybir.MatmulPerfMode

E4NP = ml_dtypes.float8_e4m3
BFNP = ml_dtypes.bfloat16

N_CORES = 8
NINP = 1024
NHID = 1024
BATCH = 16384
WSCALE = 32.0  # weight pre-scale before fp8 quantization (power of 2)

LAST_RESULTS = None  # BassKernelResults of the most recent kernel() call


def build_nc(
    K,            # input feature dim (x)
    H,            # hidden dim (y/z)
    B_shard,      # batch rows per core
    chunk,        # matmul moving-dim size (psum free size, <=512)
    wdt00, wdt10,  # W_dt scalars (baked immediates; b_dt rides in biasP)
):
    NJT = H // P           # output feature tiles (per H-sized group)
    NKT = K // P           # x contraction 128-blocks
    NHT = H // P           # y/z contraction 128-blocks
    NKP = NKT // 2         # x contraction 256-pairs (DoubleRow)
    NHP = NHT // 2
    nch = B_shard // chunk

    nc = bacc.Bacc(trn_type="TRN2", target_bir_lowering=False)

    x8 = nc.declare_dram_parameter("x8", [K // 2, 2, B_shard], FP8, isOutput=False)
    y8 = nc.declare_dram_parameter("y8", [H // 2, 2, B_shard], FP8, isOutput=False)
    ybf = nc.declare_dram_parameter("ybf", [H, B_shard], BF16, isOutput=False)
    zbf = nc.declare_dram_parameter("zbf", [H, B_shard], BF16, isOutput=False)
    dtr = nc.declare_dram_parameter("dtr", [1, B_shard], F32, isOutput=False)
    # packed stationary blocks: [jt, kin, kidx, j]; kidx 0..NKT-1 = x features,
    # NKT..NKT+NHT-1 = y (or z_new) features; values are fp8(32*W)
    NKH = NKT + NHT
    Wd2 = nc.declare_dram_parameter("Wd2", [NJT, P, NKH, P], FP8, isOutput=False)
    Wy = nc.declare_dram_parameter("Wy", [NJT, P, NKH, P], FP8, isOutput=False)
    Wd1 = nc.declare_dram_parameter("Wd1", [NJT, P, NKH, P], FP8, isOutput=False)
    Wg3 = nc.declare_dram_parameter("Wg3", [NJT, P, NKH, P], FP8, isOutput=False)
    # last two columns: row 0 holds b_dt[0], b_dt[1]
    biasP = nc.declare_dram_parameter("biasP", [P, 4 * NJT + 2], F32, isOutput=False)

    y_newT = nc.declare_dram_parameter("y_newT", [H, B_shard], F32, isOutput=True)
    z_newT = nc.declare_dram_parameter("z_newT", [H, B_shard], F32, isOutput=True)

    with tile.TileContext(nc) as tc, ExitStack() as ctx:
        dchunk = 2 * chunk          # DVE op width (2 psum chunks)
        ndc = B_shard // dchunk
        cpool = ctx.enter_context(tc.tile_pool(name="cpool", bufs=1))
        wpool = ctx.enter_context(tc.tile_pool(name="wpool", bufs=1))
        x8pool = ctx.enter_context(tc.tile_pool(name="x8pool", bufs=NKP))
        y8pool = ctx.enter_context(tc.tile_pool(name="y8pool", bufs=NHP))
        zpool = ctx.enter_context(tc.tile_pool(name="zpool", bufs=2))
        ypool = ctx.enter_context(tc.tile_pool(name="ypool", bufs=2))
        bcpool = ctx.enter_context(tc.tile_pool(name="bcpool", bufs=1))
        rpool = ctx.enter_context(tc.tile_pool(name="rpool", bufs=1))
        apool = ctx.enter_context(tc.tile_pool(name="apool", bufs=3))
        dpool = ctx.enter_context(tc.tile_pool(name="dpool", bufs=3))
        opool = ctx.enter_context(tc.tile_pool(name="opool", bufs=3))
        znpool = ctx.enter_context(tc.tile_pool(name="znpool", bufs=1))
        pspool = ctx.enter_context(tc.tile_pool(name="pspool", bufs=8, space="PSUM"))

        bias_sb = cpool.tile([P, 4 * NJT + 2], F32, name="bias_sb")
        nc.sync.dma_start(bias_sb[:], biasP[:, :])

        def bias_ap(g, jt):
            i = g * NJT + jt
            return bias_sb[:, i : i + 1]

        def cs(c):
            return slice(c * chunk, (c + 1) * chunk)

        def ds(c2):
            return slice(c2 * dchunk, (c2 + 1) * dchunk)

        # per-batch dt gates first: tiny ACT ops must precede the PSUM
        # activations in the ACT FIFO so the bc gates are ready early
        dt_sb = rpool.tile([1, B_shard], F32, name="dt_sb")
        nc.sync.dma_start(dt_sb[:], dtr[0:1, :])
        sg1 = rpool.tile([1, B_shard], BF16, name="sg1")
        nc.scalar.activation(
            sg1[:], dt_sb[:], AF.Sigmoid,
            bias=bias_sb[0:1, 4 * NJT : 4 * NJT + 1], scale=wdt00,
        )
        sg2 = rpool.tile([1, B_shard], BF16, name="sg2")
        nc.scalar.activation(
            sg2[:], dt_sb[:], AF.Sigmoid,
            bias=bias_sb[0:1, 4 * NJT + 1 : 4 * NJT + 2], scale=wdt10,
        )
        y_t = []
        for g in range(NHP):
            yt_ = y8pool.tile([P, 2, B_shard], FP8, name="yt", tag="yt")
            if g < 2:
                nc.gpsimd.dma_start(yt_[:], y8[g * P : (g + 1) * P, :, :])
            y_t.append(yt_)
        bc1 = bcpool.tile([P, B_shard], BF16, name="bc1")
        nc.gpsimd.partition_broadcast(bc1[:], sg1[0:1, :])
        bc2 = bcpool.tile([P, B_shard], BF16, name="bc2")
        nc.gpsimd.partition_broadcast(bc2[:], sg2[0:1, :])

        # ---- resident loads ----
        # phase-B weights on sync (y8 g2/g3 interleaved after jt0 weights),
        # x8 on scalar, y8 g0/g1 on gpsimd: spreads the cold-start load.
        w_d2, w_y, w_d1, w_g3 = [], [], [], []
        x_t = []
        for jt in range(NJT):
            wt = wpool.tile([P, NKH, P], FP8, name=f"wd2_{jt}", tag=f"wd2_{jt}")
            nc.sync.dma_start(wt[:], Wd2[jt][:, :, :])
            w_d2.append(wt)
            wt = wpool.tile([P, NKH, P], FP8, name=f"wy_{jt}", tag=f"wy_{jt}")
            nc.sync.dma_start(wt[:], Wy[jt][:, :, :])
            w_y.append(wt)
            if jt < NKP:
                xt_ = x8pool.tile([P, 2, B_shard], FP8, name="xt", tag="xt")
                nc.scalar.dma_start(xt_[:], x8[jt * P : (jt + 1) * P, :, :])
                x_t.append(xt_)
            if jt == 0:
                # cold-start spread: y8 g2/g3 ride the sync queue right
                # after jt0's weight blocks
                for g in (2, 3):
                    nc.sync.dma_start(y_t[g][:], y8[g * P : (g + 1) * P, :, :])
        # phase-C weights stream on gpsimd during phase B
        for jt in range(NJT):
            wt = wpool.tile([P, NKH, P], FP8, name=f"wd1_{jt}", tag=f"wd1_{jt}")
            nc.gpsimd.dma_start(wt[:], Wd1[jt][:, :, :])
            w_d1.append(wt)
            wt = wpool.tile([P, NKH, P], FP8, name=f"wg3_{jt}", tag=f"wg3_{jt}")
            nc.gpsimd.dma_start(wt[:], Wg3[jt][:, :, :])
            w_g3.append(wt)

        # fp8 z_new, resident for the W_z GEMM: [kin, hidx, batch]
        zn8 = znpool.tile([P, NHT, B_shard], FP8, name="zn8")

        def accum_group(ps_tiles, w_sb, rhs_a, rhs_b):
            """ps[c] = sum_g Wa[g].T@a[g][c] + Wb[g].T@b[g][c], DoubleRow.

            g-major / c-minor order so each stationary block is loaded once
            per nch moving matmuls."""
            n_a = len(rhs_a)
            n_b = len(rhs_b)
            for g in range(n_a):
                lhsT = w_sb[:, 2 * g : 2 * g + 2, :]
                for c in range(len(ps_tiles)):
                    nc.tensor.matmul(
                        ps_tiles[c][:], lhsT=lhsT, rhs=rhs_a[g][c],
                        start=(g == 0), stop=False, perf_mode=PM.DoubleRow,
                    )
            for g in range(n_b):
                lhsT = w_sb[:, NKT + 2 * g : NKT + 2 * g + 2, :]
                for c in range(len(ps_tiles)):
                    nc.tensor.matmul(
                        ps_tiles[c][:], lhsT=lhsT, rhs=rhs_b[g][c],
                        start=False, stop=(g == n_b - 1), perf_mode=PM.DoubleRow,
                    )

        def xrhs(g):
            return [x_t[g][:, :, cs(c)] for c in range(nch)]

        def yrhs(g):
            return [y_t[g][:, :, cs(c)] for c in range(nch)]

        # ---- phase B: d2 + y gates -> z_new ----
        for jt in range(NJT):
            jp = slice(jt * P, (jt + 1) * P)
            z16 = zpool.tile([P, B_shard], BF16, name="z16", tag="z")
            nc.gpsimd.dma_start(z16[:], zbf[jp, :])

            ps1 = [pspool.tile([P, chunk], F32, name="ps1", tag="ps") for _ in range(nch)]
            accum_group(ps1, w_d2[jt],
                        [xrhs(g) for g in range(NKP)], [yrhs(g) for g in range(NHP)])
            s2, gm = [], []
            for c2 in range(ndc):
                t = apool.tile([P, dchunk], F32, name="s2", tag="sg", bufs=2)
                nc.scalar.activation(t[:, 0:chunk], ps1[2 * c2][:], AF.Sigmoid,
                                     bias=bias_ap(0, jt), scale=1.0 / WSCALE)
                nc.scalar.activation(t[:, chunk:dchunk], ps1[2 * c2 + 1][:], AF.Sigmoid,
                                     bias=bias_ap(0, jt), scale=1.0 / WSCALE)
                s2.append(t)
                # gm only needs s2 -> issue before the second matmul sweep drains
                g_ = dpool.tile([P, dchunk], F32, name="gm", tag="gm", bufs=2)
                nc.vector.tensor_mul(g_[:], t[:], bc2[:, ds(c2)])
                gm.append(g_)

            ps2 = [pspool.tile([P, chunk], F32, name="ps2", tag="ps") for _ in range(nch)]
            accum_group(ps2, w_y[jt],
                        [xrhs(g) for g in range(NKP)], [yrhs(g) for g in range(NHP)])
            for c2 in range(ndc):
                tz = apool.tile([P, dchunk], F32, name="tz", tag="th", bufs=2)
                nc.scalar.activation(tz[:, 0:chunk], ps2[2 * c2][:], AF.Tanh,
                                     bias=bias_ap(1, jt), scale=1.0 / WSCALE)
                nc.scalar.activation(tz[:, chunk:dchunk], ps2[2 * c2 + 1][:], AF.Tanh,
                                     bias=bias_ap(1, jt), scale=1.0 / WSCALE)
                d = dpool.tile([P, dchunk], F32, name="d", tag="dm", bufs=2)
                nc.vector.tensor_sub(d[:], tz[:], z16[:, ds(c2)])
                m = dpool.tile([P, dchunk], F32, name="m", tag="mm", bufs=2)
                nc.vector.tensor_mul(m[:], gm[c2][:], d[:])
                znc = opool.tile([P, dchunk], F32, name="znc", tag="on")
                nc.vector.tensor_add(znc[:], m[:], z16[:, ds(c2)])
                nc.sync.dma_start(z_newT[jp, ds(c2)], znc[:])
                # fp8 cast into the resident zn8 for the W_z GEMM
                nc.scalar.activation(zn8[:, jt, ds(c2)], znc[:], AF.Copy)

        # ---- phase C: d1 gate + (i_z + z_new @ W_z.T) -> y_new ----
        for jt in range(NJT):
            jp = slice(jt * P, (jt + 1) * P)
            y16 = ypool.tile([P, B_shard], BF16, name="y16", tag="y")
            nc.gpsimd.dma_start(y16[:], ybf[jp, :])

            ps3 = [pspool.tile([P, chunk], F32, name="ps3", tag="ps") for _ in range(nch)]
            accum_group(ps3, w_d1[jt],
                        [xrhs(g) for g in range(NKP)], [yrhs(g) for g in range(NHP)])
            s1, gm1 = [], []
            for c2 in range(ndc):
                t = apool.tile([P, dchunk], F32, name="s1", tag="sg", bufs=2)
                nc.scalar.activation(t[:, 0:chunk], ps3[2 * c2][:], AF.Sigmoid,
                                     bias=bias_ap(2, jt), scale=1.0 / WSCALE)
                nc.scalar.activation(t[:, chunk:dchunk], ps3[2 * c2 + 1][:], AF.Sigmoid,
                                     bias=bias_ap(2, jt), scale=1.0 / WSCALE)
                s1.append(t)
                g_ = dpool.tile([P, dchunk], F32, name="gm1", tag="gm", bufs=2)
                nc.vector.tensor_mul(g_[:], t[:], bc1[:, ds(c2)])
                gm1.append(g_)

            ps4 = [pspool.tile([P, chunk], F32, name="ps4", tag="ps") for _ in range(nch)]
            accum_group(ps4, w_g3[jt],
                        [xrhs(g) for g in range(NKP)],
                        [[zn8[:, 2 * g : 2 * g + 2, cs(c)] for c in range(nch)]
                         for g in range(NHP)])
            for c2 in range(ndc):
                u = apool.tile([P, dchunk], F32, name="u", tag="th", bufs=2)
                nc.scalar.activation(u[:, 0:chunk], ps4[2 * c2][:], AF.Tanh,
                                     bias=bias_ap(3, jt), scale=1.0 / WSCALE)
                nc.scalar.activation(u[:, chunk:dchunk], ps4[2 * c2 + 1][:], AF.Tanh,
                                     bias=bias_ap(3, jt), scale=1.0 / WSCALE)
                d = dpool.tile([P, dchunk], F32, name="dy", tag="dm", bufs=2)
                nc.vector.tensor_sub(d[:], u[:], y16[:, ds(c2)])
                m = dpool.tile([P, dchunk], F32, name="my", tag="mm", bufs=2)
                nc.vector.tensor_mul(m[:], gm1[c2][:], d[:])
                yn = opool.tile([P, dchunk], F32, name="yn", tag="on")
                nc.vector.tensor_add(yn[:], m[:], y16[:, ds(c2)])
                nc.scalar.dma_start(y_newT[jp, ds(c2)], yn[:])

    nc.compile()
    return nc


def _pack_pair_fp8(Wa, Wb):
    """[jt, kin, kidx, j] stationary-block packing of two row-major [out, in]
    weight matrices, quantized to fp8(32*W)."""
    def pack(W):
        O, I = W.shape
        njt, nkt = O // P, I // P
        Wq = np.asarray(W * WSCALE, dtype=E4NP)
        # [jt, j, kt, kin] -> [jt, kin, kt, j]
        return Wq.reshape(njt, P, nkt, P).transpose(0, 3, 2, 1)
    return np.ascontiguousarray(np.concatenate([pack(Wa), pack(Wb)], axis=2))


def _pack_act_fp8(aT):
    """[K, B] fp8 -> [K//2, 2, B] DoubleRow pair-major packing."""
    Kdim, B = aT.shape
    nkp = Kdim // (2 * P)
    return np.ascontiguousarray(
        aT.reshape(nkp, 2, P, B).transpose(0, 2, 1, 3).reshape(Kdim // 2, 2, B)
    )


def pack_host_inputs(x, y, z, dt, W_ih, b_ih, W_hh, b_hh, W_z, b_z, b_dt, n_cores):
    """Shard batch across cores; quantize + pre-transpose activations;
    pack weights."""
    B, K = x.shape
    H = y.shape[1]
    NJT = H // P
    Bs = B // n_cores

    x8 = _pack_act_fp8(np.ascontiguousarray(np.asarray(x, dtype=E4NP).T))
    y8 = _pack_act_fp8(np.ascontiguousarray(np.asarray(y, dtype=E4NP).T))
    ybf = np.ascontiguousarray(np.asarray(y, dtype=BFNP).T)
    zbf = np.ascontiguousarray(np.asarray(z, dtype=BFNP).T)
    dtrow = np.ascontiguousarray(dt.reshape(1, B))

    Wd2 = _pack_pair_fp8(W_ih[H : 2 * H], W_hh[H : 2 * H])
    Wy = _pack_pair_fp8(W_ih[3 * H : 4 * H], W_hh[2 * H : 3 * H])
    Wd1 = _pack_pair_fp8(W_ih[0:H], W_hh[0:H])
    Wg3 = _pack_pair_fp8(W_ih[2 * H : 3 * H], W_z)

    def bias_cols(bvec):
        return bvec.reshape(NJT, P).T  # [P, NJT]

    bdt_cols = np.zeros((P, 2), np.float32)
    bdt_cols[0, 0] = b_dt[0]
    bdt_cols[0, 1] = b_dt[1]
    biasP = np.ascontiguousarray(
        np.concatenate(
            [
                bias_cols(b_ih[H : 2 * H] + b_hh[H : 2 * H]),
                bias_cols(b_ih[3 * H : 4 * H] + b_hh[2 * H : 3 * H]),
                bias_cols(b_ih[0:H] + b_hh[0:H]),
                bias_cols(b_ih[2 * H : 3 * H] + b_z),
                bdt_cols,
            ],
            axis=1,
        ),
        dtype=np.float32,
    )

    in_maps = []
    for c in range(n_cores):
        sl = slice(c * Bs, (c + 1) * Bs)
        in_maps.append(
            {
                "x8": np.ascontiguousarray(x8[:, :, sl]),
                "y8": np.ascontiguousarray(y8[:, :, sl]),
                "ybf": np.ascontiguousarray(ybf[:, sl]),
                "zbf": np.ascontiguousarray(zbf[:, sl]),
                "dtr": np.ascontiguousarray(dtrow[:, sl]),
                "Wd2": Wd2,
                "Wy": Wy,
                "Wd1": Wd1,
                "Wg3": Wg3,
                "biasP": biasP,
            }
        )
    return in_maps


def kernel(x, y, z, dt, W_ih, b_ih, W_hh, b_hh, W_z, b_z, W_dt, b_dt):
    x = np.asarray(x, np.float32)
    y = np.asarray(y, np.float32)
    z = np.asarray(z, np.float32)
    dt = np.asarray(dt, np.float32)
    W_ih = np.asarray(W_ih, np.float32)
    b_ih = np.asarray(b_ih, np.float32)
    W_hh = np.asarray(W_hh, np.float32)
    b_hh = np.asarray(b_hh, np.float32)
    W_z = np.asarray(W_z, np.float32)
    b_z = np.asarray(b_z, np.float32)
    W_dt = np.asarray(W_dt, np.float32)
    b_dt = np.asarray(b_dt, np.float32)

    B, K = x.shape
    H = y.shape[1]
    Bs = B // N_CORES

    in_maps = pack_host_inputs(
        x, y, z, dt, W_ih, b_ih, W_hh, b_hh, W_z, b_z, b_dt, N_CORES
    )
    nc = build_nc(
        K,
        H,
        Bs,
        chunk=512,
        wdt00=float(W_dt[0, 0]),
        wdt10=float(W_dt[1, 0]),
    )
    import os

    trace = os.environ.get("LEM_TRACE", "0") == "1"
    tmpdir = os.environ.get("LEM_TMPDIR") or None
    res = run_bass_kernel_spmd(
        nc, in_maps, list(range(N_CORES)), trace=trace, tmpdir=tmpdir
    )
    global LAST_RESULTS
    LAST_RESULTS = res
    y_newT = np.concatenate([np.asarray(r["y_newT"]) for r in res.results], axis=1)
    z_newT = np.concatenate([np.asarray(r["z_newT"]) for r in res.results], axis=1)
    return (
        np.ascontiguousarray(y_newT.T, dtype=np.float32),
        np.ascontiguousarray(z_newT.T, dtype=np.float32),
    )
